# revision 30
# baseline (speedup 1.0000x reference)
"""Trainium2 Bass kernel for MQA causal attention (16 q heads, 1 shared kv head).

Sharding: hybrid batch x tensor-parallel. Core c handles batch c//4 and query
heads [4*(c%4), 4*(c%4)+4) (4 heads per core), shared K/V computed per batch
group (4x replication instead of 8x). Each core emits a bf16 partial
out-projection for its batch; the host sums the 4 partials per batch (the
all-reduce of the hint).

Per-core layout:
  - x arrives dim-major (xT, bf16): every matmul contraction dim is already
    on partitions; no on-chip transposes of x.
  - Projections: psq/psk/psv accumulate over DC=16 chunks in PSUM, then are
    copied to SBUF bf16 on ACT; RoPE runs on DVE fully in bf16 (2x mode):
    q_rot = q*cos + rot(q)*sin with rot done by partition-offset reads and
    host-pre-signed sin tables. q tables pre-scaled by 1/sqrt(d).
  - Attention is computed transposed at 4-head width: simT[keys, h*q] =
    kT.T @ qT per 128-key chunk, 2 matmuls (head pairs) so every matmul
    output stays within one PSUM bank. exp on ACT ([128,1024] per op),
    causal masking only on the two diagonal key chunks via affine_select,
    denominator = ones-column matmuls accumulated in PSUM, attn@V keeps V
    natural [keys, d] (PE-transposed at projection time) accumulating
    psa[d, h*q] in PSUM.
  - psa is evicted UN-normalized (frees the single psa PSUM ring slot
    early); normalization happens in-place on the bf16 tile after a
    reciprocal + partition_broadcast of the denominators.
  - Out-projection: attnT chunks stationary, Wout slice moving; psy evicted
    to bf16 ysb split across DVE and ACT; y written bf16.
  - Emission interleaves projection slices, attention tiles and
    out-projection chunks so the PE stream fills ACT-wait gaps.
"""

import os
import sys
from contextlib import ExitStack

import numpy as np

for _p in ("/opt/trn_rl_repo",):
    if os.path.isdir(_p) and _p not in sys.path:
        sys.path.insert(0, _p)

import ml_dtypes

import concourse.bass as bass
import concourse.mybir as mybir
import concourse.tile as tile
from concourse import bacc
from concourse.bass_utils import run_bass_kernel_spmd
from concourse.masks import make_identity

HEADS = 16
D = 128
SCALE = D ** -0.5
N_CORES = 8
HL = 4                      # query heads per core
GROUPS = 4                  # cores per batch group

F32 = mybir.dt.float32
BF16 = mybir.dt.bfloat16


def _rope(nc, sb, src, out_slice, cos_s, sin_s):
    """out_slice(bf16) = src*cos_s + rot(src)*sin_s, all bf16 on DVE (2x).

    sin_s arrives pre-signed AND pre-rotated from the host (halves swapped,
    rows that multiply the swapped-in half negated) so that both DVE inputs
    always share the same base partition (a same-space DVE requirement)."""
    L = src.shape[-1]
    t1 = sb.tile([128, L], BF16, tag="ropet1")
    nc.vector.tensor_mul(t1, src, cos_s)
    t2 = sb.tile([128, L], BF16, tag="ropet2")
    nc.vector.tensor_mul(t2[0:64, :], src[64:128, :], sin_s[64:128, :])
    nc.vector.tensor_mul(t2[64:128, :], src[0:64, :], sin_s[0:64, :])
    nc.vector.tensor_add(out_slice, t1, t2)


def build_nc(N, DIM, reps=1):
    """One SPMD program: HL query heads + shared kv head, one batch,
    full sequence. reps>1 repeats the body for timing-by-difference."""
    DC = DIM // 128           # contraction chunks for projections
    SL = 512                  # projection n-slice length
    NS = N // SL              # n slices (4)
    NKC = N // 128            # 128-wide key chunks (16)
    NQT = N // 256            # 256-row query tiles (8)
    KPS = SL // 128           # key chunks per slice (4)

    nc = bacc.Bacc(None, target_bir_lowering=False)
    xT = nc.declare_dram_parameter("xT", [DIM, N], BF16, isOutput=False)
    # this core's kv shard: xkv = xT[:, hg*512:(hg+1)*512] host-sliced, so
    # the SPMD program stays core-id independent; k rope tables likewise
    xkv = nc.declare_dram_parameter("xkv", [DIM, SL], BF16, isOutput=False)
    ckv = nc.declare_dram_parameter("ckv", [D, SL], BF16, isOutput=False)
    skv = nc.declare_dram_parameter("skv", [D, SL], BF16, isOutput=False)
    wq = nc.declare_dram_parameter("wq", [DIM, HL * D], BF16, isOutput=False)
    wkv = nc.declare_dram_parameter("wkv", [DIM, 2 * D], BF16, isOutput=False)
    wout = nc.declare_dram_parameter("wout", [HL * D, DIM], BF16, isOutput=False)
    cosq = nc.declare_dram_parameter("cosq", [D, N], BF16, isOutput=False)
    sinq = nc.declare_dram_parameter("sinq", [D, N], BF16, isOutput=False)
    y = nc.declare_dram_parameter("y", [N, DIM], BF16, isOutput=True)
    REPLICA_GROUPS = [[0, 1, 2, 3], [4, 5, 6, 7]]

    with ExitStack() as ctx:
        tc = ctx.enter_context(tile.TileContext(nc))
        consts = ctx.enter_context(tc.tile_pool(name="consts", bufs=1))
        xpool = ctx.enter_context(tc.tile_pool(name="xpool", bufs=2))
        proj = ctx.enter_context(tc.tile_pool(name="proj", bufs=2))
        sb = ctx.enter_context(tc.tile_pool(name="sb", bufs=2))
        misc = ctx.enter_context(tc.tile_pool(name="misc", bufs=2))
        dram = ctx.enter_context(tc.tile_pool(name="dram", bufs=2, space="DRAM"))
        # PSUM: stream ring 3 x [128,1024]f32 (6 banks) for all transient
        # psums (sim chunks, proj groups, outproj psy, per-tile psd) + psa
        # ring 1 (2 banks) = 8 banks.
        ps_str = ctx.enter_context(tc.tile_pool(name="ps_str", bufs=3, space="PSUM"))
        ps_acc = ctx.enter_context(tc.tile_pool(name="ps_acc", bufs=1, space="PSUM"))

        ident = consts.tile([128, 128], BF16)
        make_identity(nc, ident)
        ones_col = consts.tile([128, 1], BF16)
        nc.vector.memset(ones_col, 1.0)

        # only SP (sync) and ACT (scalar) have HWDGE queues; order for ramp:
        # sync: wkv, x slice 0, k tables, x slices 1-3
        # scalar: wq, q tables, wout, then y writes
        wq_sb = consts.tile([128, DC, HL * D], BF16)
        wkv_sb = consts.tile([128, DC, 2 * D], BF16)
        nc.sync.dma_start(wkv_sb, wkv.rearrange("(c p) m -> p c m", p=128))
        nc.scalar.dma_start(wq_sb, wq.rearrange("(c p) m -> p c m", p=128))
        cq_sb = consts.tile([128, N], BF16)
        sq_sb = consts.tile([128, N], BF16)
        ckv_sb = consts.tile([128, SL], BF16)
        skv_sb = consts.tile([128, SL], BF16)
        nc.sync.dma_start(ckv_sb, ckv[:, :])
        nc.sync.dma_start(skv_sb, skv[:, :])
        wout_sb = consts.tile([128, HL, DIM], BF16)

        tables_loaded = [False]

        def _load_tables():
            # emitted after x slice 0's dma so the slice-0 stream goes first
            nc.scalar.dma_start(cq_sb, cosq[:, :])
            nc.scalar.dma_start(sq_sb, sinq[:, :])
            nc.scalar.dma_start(
                wout_sb, wout.rearrange("(c p) m -> p c m", p=128))
            tables_loaded[0] = True

        def _kv_launch():
            """Compute this core's quarter of k/v (+rope/transpose), stage
            to DRAM and trigger the 4-core AllGather. Returns the gathered
            DRAM tile. Stage-out + trigger ride the Pool queue."""
            xkv_sb = xpool.tile([128, DC, SL], BF16, tag="xt")
            xkv_src = xkv.rearrange("(c p) n -> p c n", p=128)
            nc.sync.dma_start(xkv_sb[:, :DC // 2, :], xkv_src[:, :DC // 2, :])
            nc.sync.dma_start(xkv_sb[:, DC // 2:, :], xkv_src[:, DC // 2:, :])
            psv = ps_str.tile([128, SL], F32, tag="stream")
            for dc in range(DC):
                nc.tensor.matmul(
                    psv, wkv_sb[:, dc, D:2 * D], xkv_sb[:, dc, :],
                    start=(dc == 0), stop=(dc == DC - 1))
            vt_sb = sb.tile([128, SL], BF16, tag="vt")
            nc.scalar.copy(vt_sb, psv)
            psk = ps_str.tile([128, SL], F32, tag="stream")
            for dc in range(DC):
                nc.tensor.matmul(
                    psk, wkv_sb[:, dc, 0:D], xkv_sb[:, dc, :],
                    start=(dc == 0), stop=(dc == DC - 1))
            ks = sb.tile([128, SL], BF16, tag="ks")
            nc.scalar.copy(ks, psk)
            kloc = sb.tile([128, SL], BF16, tag="kloc")
            _rope(nc, sb, ks, kloc, ckv_sb, skv_sb)
            vloc = sb.tile([128, KPS, D], BF16, tag="vloc")
            for kc in range(KPS):
                pst = ps_str.tile([128, 128], BF16, tag="stream")
                nc.tensor.transpose(pst, vt_sb[:, kc * 128:(kc + 1) * 128], ident)
                nc.vector.tensor_copy(vloc[:, kc, :], pst)
            kv_loc = dram.tile([128, 2 * SL], BF16, tag="kv_loc")
            kv_all = dram.tile([GROUPS, 128, 2 * SL], BF16, tag="kv_all")
            nc.gpsimd.dma_start(kv_loc[:, 0:SL], kloc)
            nc.gpsimd.dma_start(kv_loc[:, SL:2 * SL], vloc)
            nc.gpsimd.collective_compute(
                "AllGather", mybir.AluOpType.bypass, REPLICA_GROUPS,
                ins=[kv_loc[:, :]], outs=[kv_all[:, :, :]])
            return kv_all

        def _kv_land(kv_all, krot, vnat):
            """Gathered k/v -> SBUF. On the sync queue, emitted LATE in the
            prior rep so the collective has completed and never blocks the
            queue head in front of the next rep's x-tile prefetches."""
            for g in range(GROUPS):
                nc.sync.dma_start(
                    krot[:, g * SL:(g + 1) * SL], kv_all[g, :, 0:SL])
                nc.sync.dma_start(
                    vnat[:, g * KPS:(g + 1) * KPS, :],
                    kv_all[g, :, SL:2 * SL])

        pending = None   # (kv_all, krot_next, vnat_next) launched a rep ago

        for rep in range(reps):
            first = rep == 0
            qrot = proj.tile([128, HL, N], BF16, tag="qrot")
            attnT = proj.tile([128, HL, N], BF16, tag="attnT")
            if pending is None:
                krot = proj.tile([128, N], BF16, tag="krot")
                vnat = proj.tile([128, NKC, D], BF16, tag="vnat")
                _kv_land(_kv_launch(), krot, vnat)
            else:
                _, krot, vnat = pending
            if rep + 1 < reps:
                kv_all2 = _kv_launch()
                krot2 = proj.tile([128, N], BF16, tag="krot")
                vnat2 = proj.tile([128, NKC, D], BF16, tag="vnat")
                pending = (kv_all2, krot2, vnat2)
            else:
                pending = None

            def _proj(s, qrot=qrot, first=first):
                sl = slice(s * SL, (s + 1) * SL)
                xt = xpool.tile([128, DC, SL], BF16, tag="xt")
                h_dc = DC // 2
                xt_src = xT.rearrange("(c p) n -> p c n", p=128)[:, :, sl]
                nc.sync.dma_start(xt[:, :h_dc, :], xt_src[:, :h_dc, :])
                nc.sync.dma_start(xt[:, h_dc:, :], xt_src[:, h_dc:, :])
                if not tables_loaded[0]:
                    _load_tables()
                for h in range(HL):
                    psq = ps_str.tile([128, SL], F32, tag="stream")
                    for dc in range(DC):
                        nc.tensor.matmul(
                            psq, wq_sb[:, dc, h * D:(h + 1) * D], xt[:, dc, :],
                            start=(dc == 0), stop=(dc == DC - 1))
                    qs = sb.tile([128, SL], BF16, tag="qs")
                    nc.scalar.copy(qs, psq)
                    _rope(nc, sb, qs, qrot[:, h, sl], cq_sb[:, sl], sq_sb[:, sl])

            # outproj work queue: each entry is a closure emitting ~0.9us of
            # PE work (half a psy group); drained one per attention chunk so
            # the out-projection fills the PE slack while ACT runs exp.
            opq = []

            def _outproj_quanta(t, attnT=attnT):
                for m in (2 * t, 2 * t + 1):
                    for nso in range(DIM // 1024):
                        state = {}

                        def q1(m=m, nso=nso, state=state):
                            psy = ps_str.tile([128, 1024], F32, tag="stream",
                                              name="psy")
                            state["psy"] = psy
                            for hc in range(HL):
                                nc.tensor.matmul(
                                    psy[:, 0:512],
                                    attnT[:, hc, m * 128:(m + 1) * 128],
                                    wout_sb[:, hc,
                                            nso * 1024:nso * 1024 + 512],
                                    start=(hc == 0), stop=(hc == HL - 1))

                        def q2(m=m, nso=nso, state=state):
                            psy = state["psy"]
                            for hc in range(HL):
                                nc.tensor.matmul(
                                    psy[:, 512:1024],
                                    attnT[:, hc, m * 128:(m + 1) * 128],
                                    wout_sb[:, hc,
                                            nso * 1024 + 512:(nso + 1) * 1024],
                                    start=(hc == 0), stop=(hc == HL - 1))
                            ysb = misc.tile([128, 1024], BF16, tag="ysb")
                            if nso % 2 == 0:
                                nc.vector.tensor_copy(ysb, psy)
                            else:
                                nc.scalar.copy(ysb, psy)
                            nc.scalar.dma_start(
                                y[m * 128:(m + 1) * 128,
                                  nso * 1024:(nso + 1) * 1024], ysb)

                        yield q1
                        yield q2

            def _attn(t, qrot=qrot, krot=krot, vnat=vnat, attnT=attnT):
                nkc = 2 * t + 2
                psa = ps_acc.tile([128, HL, 256], F32, tag="psa")
                exacc = sb.tile([128, HL, 256], BF16, tag="exacc")
                qsl = qrot[:, :, t * 256:(t + 1) * 256]
                # diagonal chunks FIRST: their Pool mask latency then hides
                # behind the long run of mask-free chunks instead of stalling
                # the PE at the tile boundary.  attnv of chunk j is emitted
                # AFTER sim of chunk j+1 (software pipelining) so the
                # in-order PE stream never head-of-line blocks on ACT's exp.
                order = [2 * t, 2 * t + 1] + list(range(0, 2 * t))

                def _consume(ci, j, ex):
                    st, sp = ci == 0, ci == nkc - 1
                    nc.tensor.matmul(psa[:, 0:2, :], vnat[:, j, :], ex[:, 0:2, :],
                                     start=st, stop=sp)
                    nc.tensor.matmul(psa[:, 2:4, :], vnat[:, j, :], ex[:, 2:4, :],
                                     start=st, stop=sp)
                    # denominator accumulation rides DVE (bf16 2x), not PE
                    if ci == 0:
                        nc.vector.tensor_copy(exacc, ex)
                    else:
                        nc.vector.tensor_add(exacc, exacc, ex)

                prev = None
                for ci, j in enumerate(order):
                    kj = krot[:, j * 128:(j + 1) * 128]
                    pss = ps_str.tile([128, HL, 256], F32, tag="stream")
                    nc.tensor.matmul(pss[:, 0:2, :], kj, qsl[:, 0:2, :],
                                     start=True, stop=True)
                    nc.tensor.matmul(pss[:, 2:4, :], kj, qsl[:, 2:4, :],
                                     start=True, stop=True)
                    ex = sb.tile([128, HL, 256], BF16, tag="ex", bufs=3)
                    nc.scalar.activation(ex, pss, mybir.ActivationFunctionType.Exp)
                    if j >= 2 * t:
                        # diagonal chunk: keep where q - p - base >= 0
                        nc.gpsimd.affine_select(
                            out=ex, in_=ex,
                            compare_op=mybir.AluOpType.is_ge, fill=0.0,
                            base=(0 if j == 2 * t else -128),
                            pattern=[[0, HL], [1, 256]],
                            channel_multiplier=-1)
                    if prev is not None:
                        _consume(*prev)
                    prev = (ci, j, ex)
                    if ci >= 2 and opq:
                        opq.pop(0)()
                _consume(*prev)
                # denominator: one PE reduction of exacc per tile, into a
                # transient stream-slot psd; then recip+broadcast+normalize
                psd = ps_str.tile([1, HL, 256], F32, tag="stream", name="psd")
                nc.tensor.matmul(psd[:, 0:2, :], ones_col, exacc[:, 0:2, :],
                                 start=True, stop=True)
                nc.tensor.matmul(psd[:, 2:4, :], ones_col, exacc[:, 2:4, :],
                                 start=True, stop=True)
                # evict unnormalized (frees psa ring slot), normalize in place
                asl = attnT[:, :, t * 256:(t + 1) * 256]
                nc.vector.tensor_copy(asl, psa)
                rec = misc.tile([1, HL, 256], BF16, tag="rec")
                with nc.allow_low_precision("softmax recip in bf16 is ~0.1%"):
                    nc.vector.reciprocal(rec, psd)
                bc = misc.tile([128, HL, 256], BF16, tag="bc")
                nc.gpsimd.partition_broadcast(bc, rec)
                nc.vector.tensor_mul(asl, asl, bc)

            # proj slices feed attention tiles; outproj quanta of tile t are
            # queued when attn(t+1) starts and drained inside the chunk loops
            _proj(0)
            _attn(0)
            _proj(1)
            _attn(1)
            opq.extend(_outproj_quanta(0))
            _attn(2)
            opq.extend(_outproj_quanta(1))
            _proj(2)
            _attn(3)
            opq.extend(_outproj_quanta(2))
            _attn(4)
            opq.extend(_outproj_quanta(3))
            _proj(3)
            _attn(5)
            opq.extend(_outproj_quanta(4))
            _attn(6)
            if pending is not None:
                # land next rep's gathered k/v now: the collective (launched
                # at this rep's start) is long done, so these never block
                # the sync queue
                _kv_land(*pending)
            opq.extend(_outproj_quanta(5))
            _attn(7)
            opq.extend(_outproj_quanta(6))
            opq.extend(_outproj_quanta(7))
            while opq:
                opq.pop(0)()

    nc.finalize()
    return nc


def make_host_inputs(x, Wq, Wkv, Wout):
    """Shard + precompute per-core input maps (host side)."""
    B, N, DIM = x.shape
    bf = ml_dtypes.bfloat16
    xTb = [np.ascontiguousarray(x[b].T).astype(bf) for b in range(B)]
    inv = 1.0 / (10000.0 ** (np.arange(0, D, 2, dtype=np.float64) / D))
    fr = np.arange(N, dtype=np.float64)[:, None] * inv[None, :]
    pos = np.concatenate([fr, fr], axis=-1)              # [N, D]
    cos_t = np.cos(pos).T.astype(np.float32)             # [D, N]
    sin_t = np.sin(pos).T.astype(np.float32)
    sign = np.ones((D, 1), np.float32)
    sign[:D // 2] = -1.0
    sin_r = sin_t * sign            # fold rotate_half's sign into the table
    # pre-rotate: row p holds sin_signed[(p+64)%128] so the kernel's
    # same-base-partition reads line up (see _rope)
    sin_r = np.roll(sin_r, -D // 2, axis=0)
    SL = N // GROUPS
    shared = dict(
        wkv=Wkv.astype(bf),
        cosq=np.ascontiguousarray(cos_t * SCALE).astype(bf),
        sinq=np.ascontiguousarray(sin_r * SCALE).astype(bf))
    cosk_b = cos_t.astype(bf)
    sink_b = sin_r.astype(bf)
    in_maps = []
    for c in range(N_CORES):
        b = c // GROUPS
        hg = c % GROUPS
        lo, hi = hg * HL * D, (hg + 1) * HL * D
        ksl = slice(hg * SL, (hg + 1) * SL)
        in_maps.append(dict(
            shared,
            xT=xTb[b],
            xkv=np.ascontiguousarray(xTb[b][:, ksl]),
            ckv=np.ascontiguousarray(cosk_b[:, ksl]),
            skv=np.ascontiguousarray(sink_b[:, ksl]),
            wq=np.ascontiguousarray(Wq[:, lo:hi]).astype(bf),
            wout=np.ascontiguousarray(Wout[lo:hi, :]).astype(bf)))
    return in_maps


def kernel(x, Wq, Wkv, Wout):
    B, N, DIM = x.shape
    nc = build_nc(N, DIM)
    in_maps = make_host_inputs(x, Wq, Wkv, Wout)
    res = run_bass_kernel_spmd(nc, in_maps, core_ids=list(range(N_CORES)))
    y = np.zeros((B, N, DIM), np.float32)
    for c, r in enumerate(res.results):
        y[c // GROUPS] += r["y"].astype(np.float32)
    return y


# revision 31
# speedup vs baseline: 1.0360x; 1.0360x over previous
"""Trainium2 Bass kernel for MQA causal attention (16 q heads, 1 shared kv head).

Sharding: hybrid batch x tensor-parallel. Core c handles batch c//4 and query
heads [4*(c%4), 4*(c%4)+4) (4 heads per core), shared K/V computed per batch
group (4x replication instead of 8x). Each core emits a bf16 partial
out-projection for its batch; the host sums the 4 partials per batch (the
all-reduce of the hint).

Per-core layout:
  - x arrives dim-major (xT, bf16): every matmul contraction dim is already
    on partitions; no on-chip transposes of x.
  - Projections: psq/psk/psv accumulate over DC=16 chunks in PSUM, then are
    copied to SBUF bf16 on ACT; RoPE runs on DVE fully in bf16 (2x mode):
    q_rot = q*cos + rot(q)*sin with rot done by partition-offset reads and
    host-pre-signed sin tables. q tables pre-scaled by 1/sqrt(d).
  - Attention is computed transposed at 4-head width: simT[keys, h*q] =
    kT.T @ qT per 128-key chunk, 2 matmuls (head pairs) so every matmul
    output stays within one PSUM bank. exp on ACT ([128,1024] per op),
    causal masking only on the two diagonal key chunks via affine_select,
    denominator = ones-column matmuls accumulated in PSUM, attn@V keeps V
    natural [keys, d] (PE-transposed at projection time) accumulating
    psa[d, h*q] in PSUM.
  - psa is evicted UN-normalized (frees the single psa PSUM ring slot
    early); normalization happens in-place on the bf16 tile after a
    reciprocal + partition_broadcast of the denominators.
  - Out-projection: attnT chunks stationary, Wout slice moving; psy evicted
    to bf16 ysb split across DVE and ACT; y written bf16.
  - Emission interleaves projection slices, attention tiles and
    out-projection chunks so the PE stream fills ACT-wait gaps.
"""

import os
import sys
from contextlib import ExitStack

import numpy as np

for _p in ("/opt/trn_rl_repo",):
    if os.path.isdir(_p) and _p not in sys.path:
        sys.path.insert(0, _p)

import ml_dtypes

import concourse.bass as bass
import concourse.mybir as mybir
import concourse.tile as tile
from concourse import bacc
from concourse.bass_utils import run_bass_kernel_spmd
from concourse.masks import make_identity

HEADS = 16
D = 128
SCALE = D ** -0.5
N_CORES = 8
HL = 4                      # query heads per core
GROUPS = 4                  # cores per batch group

F32 = mybir.dt.float32
BF16 = mybir.dt.bfloat16


def _rope(nc, sb, src, out_slice, cos_s, sin_s):
    """out_slice(bf16) = src*cos_s + rot(src)*sin_s, all bf16 on DVE (2x).

    sin_s arrives pre-signed AND pre-rotated from the host (halves swapped,
    rows that multiply the swapped-in half negated) so that both DVE inputs
    always share the same base partition (a same-space DVE requirement)."""
    L = src.shape[-1]
    t1 = sb.tile([128, L], BF16, tag="ropet1")
    nc.vector.tensor_mul(t1, src, cos_s)
    t2 = sb.tile([128, L], BF16, tag="ropet2")
    nc.vector.tensor_mul(t2[0:64, :], src[64:128, :], sin_s[64:128, :])
    nc.vector.tensor_mul(t2[64:128, :], src[0:64, :], sin_s[0:64, :])
    nc.vector.tensor_add(out_slice, t1, t2)


def build_nc(N, DIM, reps=1):
    """One SPMD program: HL query heads + shared kv head, one batch,
    full sequence. reps>1 repeats the body for timing-by-difference."""
    DC = DIM // 128           # contraction chunks for projections
    SL = 512                  # projection n-slice length
    NS = N // SL              # n slices (4)
    NKC = N // 128            # 128-wide key chunks (16)
    NQT = N // 256            # 256-row query tiles (8)
    KPS = SL // 128           # key chunks per slice (4)

    nc = bacc.Bacc(None, target_bir_lowering=False)
    xT = nc.declare_dram_parameter("xT", [DIM, N], BF16, isOutput=False)
    wq = nc.declare_dram_parameter("wq", [DIM, HL * D], BF16, isOutput=False)
    wkv = nc.declare_dram_parameter("wkv", [DIM, 2 * D], BF16, isOutput=False)
    wout = nc.declare_dram_parameter("wout", [HL * D, DIM], BF16, isOutput=False)
    cosq = nc.declare_dram_parameter("cosq", [D, N], BF16, isOutput=False)
    sinq = nc.declare_dram_parameter("sinq", [D, N], BF16, isOutput=False)
    cosk = nc.declare_dram_parameter("cosk", [D, N], BF16, isOutput=False)
    sink = nc.declare_dram_parameter("sink", [D, N], BF16, isOutput=False)
    y = nc.declare_dram_parameter("y", [N, DIM], BF16, isOutput=True)

    with ExitStack() as ctx:
        tc = ctx.enter_context(tile.TileContext(nc))
        consts = ctx.enter_context(tc.tile_pool(name="consts", bufs=1))
        xpool = ctx.enter_context(tc.tile_pool(name="xpool", bufs=2))
        proj = ctx.enter_context(tc.tile_pool(name="proj", bufs=2))
        sb = ctx.enter_context(tc.tile_pool(name="sb", bufs=2))
        misc = ctx.enter_context(tc.tile_pool(name="misc", bufs=2))
        # PSUM: stream ring 3 x [128,1024]f32 (6 banks) for all transient
        # psums (sim chunks, proj groups, outproj psy, per-tile psd) + psa
        # ring 1 (2 banks) = 8 banks.
        ps_str = ctx.enter_context(tc.tile_pool(name="ps_str", bufs=3, space="PSUM"))
        ps_acc = ctx.enter_context(tc.tile_pool(name="ps_acc", bufs=1, space="PSUM"))

        ident = consts.tile([128, 128], BF16)
        make_identity(nc, ident)
        ones_col = consts.tile([128, 1], BF16)
        nc.vector.memset(ones_col, 1.0)

        # only SP (sync) and ACT (scalar) have HWDGE queues; order for ramp:
        # sync: wkv, x slice 0, k tables, x slices 1-3
        # scalar: wq, q tables, wout, then y writes
        wq_sb = consts.tile([128, DC, HL * D], BF16)
        wkv_sb = consts.tile([128, DC, 2 * D], BF16)
        nc.sync.dma_start(wkv_sb, wkv.rearrange("(c p) m -> p c m", p=128))
        nc.scalar.dma_start(wq_sb, wq.rearrange("(c p) m -> p c m", p=128))
        cq_sb = consts.tile([128, N], BF16)
        sq_sb = consts.tile([128, N], BF16)
        ck_sb = consts.tile([128, N], BF16)
        sk_sb = consts.tile([128, N], BF16)
        wout_sb = consts.tile([128, HL, DIM], BF16)

        tables_loaded = [False]

        def _load_tables():
            # emitted after x slice 0's dma so the slice-0 stream goes first
            nc.sync.dma_start(ck_sb, cosk[:, :])
            nc.sync.dma_start(sk_sb, sink[:, :])
            nc.scalar.dma_start(cq_sb, cosq[:, :])
            nc.scalar.dma_start(sq_sb, sinq[:, :])
            nc.scalar.dma_start(
                wout_sb, wout.rearrange("(c p) m -> p c m", p=128))
            tables_loaded[0] = True

        for rep in range(reps):
            first = rep == 0
            qrot = proj.tile([128, HL, N], BF16, tag="qrot")
            krot = proj.tile([128, N], BF16, tag="krot")
            vnat = proj.tile([128, NKC, D], BF16, tag="vnat")
            attnT = proj.tile([128, HL, N], BF16, tag="attnT")

            def _proj(s, qrot=qrot, krot=krot, vnat=vnat, first=first):
                sl = slice(s * SL, (s + 1) * SL)
                xt = xpool.tile([128, DC, SL], BF16, tag="xt")
                h_dc = DC // 2
                xt_src = xT.rearrange("(c p) n -> p c n", p=128)[:, :, sl]
                nc.sync.dma_start(xt[:, :h_dc, :], xt_src[:, :h_dc, :])
                nc.sync.dma_start(xt[:, h_dc:, :], xt_src[:, h_dc:, :])
                if not tables_loaded[0]:
                    _load_tables()
                # v first: its psum->sbuf copy rides ACT early
                psv = ps_str.tile([128, SL], F32, tag="stream")
                for dc in range(DC):
                    nc.tensor.matmul(
                        psv, wkv_sb[:, dc, D:2 * D], xt[:, dc, :],
                        start=(dc == 0), stop=(dc == DC - 1))
                vt_sb = sb.tile([128, SL], BF16, tag="vt")
                nc.scalar.copy(vt_sb, psv)
                # k next so attention tiles unblock asap
                psk = ps_str.tile([128, SL], F32, tag="stream")
                for dc in range(DC):
                    nc.tensor.matmul(
                        psk, wkv_sb[:, dc, 0:D], xt[:, dc, :],
                        start=(dc == 0), stop=(dc == DC - 1))
                ks = sb.tile([128, SL], BF16, tag="ks")
                nc.scalar.copy(ks, psk)
                _rope(nc, sb, ks, krot[:, sl], ck_sb[:, sl], sk_sb[:, sl])
                for h in range(HL):
                    psq = ps_str.tile([128, SL], F32, tag="stream")
                    for dc in range(DC):
                        nc.tensor.matmul(
                            psq, wq_sb[:, dc, h * D:(h + 1) * D], xt[:, dc, :],
                            start=(dc == 0), stop=(dc == DC - 1))
                    qs = sb.tile([128, SL], BF16, tag="qs")
                    nc.scalar.copy(qs, psq)
                    _rope(nc, sb, qs, qrot[:, h, sl], cq_sb[:, sl], sq_sb[:, sl])
                for kc in range(KPS):
                    pst = ps_str.tile([128, 128], BF16, tag="stream")
                    nc.tensor.transpose(pst, vt_sb[:, kc * 128:(kc + 1) * 128], ident)
                    nc.vector.tensor_copy(vnat[:, s * KPS + kc, :], pst)

            # outproj work queue: each entry is a closure emitting ~0.9us of
            # PE work (half a psy group); drained one per attention chunk so
            # the out-projection fills the PE slack while ACT runs exp.
            opq = []

            def _outproj_quanta(t, attnT=attnT):
                for m in (2 * t, 2 * t + 1):
                    for nso in range(DIM // 1024):
                        state = {}

                        def q1(m=m, nso=nso, state=state):
                            psy = ps_str.tile([128, 1024], F32, tag="stream",
                                              name="psy")
                            state["psy"] = psy
                            for hc in range(HL):
                                nc.tensor.matmul(
                                    psy[:, 0:512],
                                    attnT[:, hc, m * 128:(m + 1) * 128],
                                    wout_sb[:, hc,
                                            nso * 1024:nso * 1024 + 512],
                                    start=(hc == 0), stop=(hc == HL - 1))

                        def q2(m=m, nso=nso, state=state):
                            psy = state["psy"]
                            for hc in range(HL):
                                nc.tensor.matmul(
                                    psy[:, 512:1024],
                                    attnT[:, hc, m * 128:(m + 1) * 128],
                                    wout_sb[:, hc,
                                            nso * 1024 + 512:(nso + 1) * 1024],
                                    start=(hc == 0), stop=(hc == HL - 1))
                            ysb = misc.tile([128, 1024], BF16, tag="ysb")
                            if nso % 2 == 0:
                                nc.vector.tensor_copy(ysb, psy)
                            else:
                                nc.scalar.copy(ysb, psy)
                            nc.scalar.dma_start(
                                y[m * 128:(m + 1) * 128,
                                  nso * 1024:(nso + 1) * 1024], ysb)

                        yield q1
                        yield q2

            def _attn(t, qrot=qrot, krot=krot, vnat=vnat, attnT=attnT):
                nkc = 2 * t + 2
                psa = ps_acc.tile([128, HL, 256], F32, tag="psa")
                exacc = sb.tile([128, HL, 256], BF16, tag="exacc")
                qsl = qrot[:, :, t * 256:(t + 1) * 256]
                # diagonal chunks FIRST: their Pool mask latency then hides
                # behind the long run of mask-free chunks instead of stalling
                # the PE at the tile boundary.  attnv of chunk j is emitted
                # AFTER sim of chunk j+1 (software pipelining) so the
                # in-order PE stream never head-of-line blocks on ACT's exp.
                order = [2 * t, 2 * t + 1] + list(range(0, 2 * t))

                def _consume(ci, j, ex):
                    st, sp = ci == 0, ci == nkc - 1
                    nc.tensor.matmul(psa[:, 0:2, :], vnat[:, j, :], ex[:, 0:2, :],
                                     start=st, stop=sp)
                    nc.tensor.matmul(psa[:, 2:4, :], vnat[:, j, :], ex[:, 2:4, :],
                                     start=st, stop=sp)
                    # denominator accumulation rides DVE (bf16 2x), not PE
                    if ci == 0:
                        nc.vector.tensor_copy(exacc, ex)
                    else:
                        nc.vector.tensor_add(exacc, exacc, ex)

                prev = None
                for ci, j in enumerate(order):
                    kj = krot[:, j * 128:(j + 1) * 128]
                    pss = ps_str.tile([128, HL, 256], F32, tag="stream")
                    nc.tensor.matmul(pss[:, 0:2, :], kj, qsl[:, 0:2, :],
                                     start=True, stop=True)
                    nc.tensor.matmul(pss[:, 2:4, :], kj, qsl[:, 2:4, :],
                                     start=True, stop=True)
                    ex = sb.tile([128, HL, 256], BF16, tag="ex", bufs=3)
                    nc.scalar.activation(ex, pss, mybir.ActivationFunctionType.Exp)
                    if j >= 2 * t:
                        # diagonal chunk: keep where q - p - base >= 0
                        nc.gpsimd.affine_select(
                            out=ex, in_=ex,
                            compare_op=mybir.AluOpType.is_ge, fill=0.0,
                            base=(0 if j == 2 * t else -128),
                            pattern=[[0, HL], [1, 256]],
                            channel_multiplier=-1)
                    if prev is not None:
                        _consume(*prev)
                    prev = (ci, j, ex)
                    if ci >= 2 and opq:
                        opq.pop(0)()
                _consume(*prev)
                # denominator: one PE reduction of exacc per tile, into a
                # transient stream-slot psd; then recip+broadcast+normalize
                psd = ps_str.tile([1, HL, 256], F32, tag="stream", name="psd")
                nc.tensor.matmul(psd[:, 0:2, :], ones_col, exacc[:, 0:2, :],
                                 start=True, stop=True)
                nc.tensor.matmul(psd[:, 2:4, :], ones_col, exacc[:, 2:4, :],
                                 start=True, stop=True)
                # evict unnormalized (frees psa ring slot), normalize in place
                asl = attnT[:, :, t * 256:(t + 1) * 256]
                nc.vector.tensor_copy(asl, psa)
                rec = misc.tile([1, HL, 256], BF16, tag="rec")
                with nc.allow_low_precision("softmax recip in bf16 is ~0.1%"):
                    nc.vector.reciprocal(rec, psd)
                bc = misc.tile([128, HL, 256], BF16, tag="bc")
                nc.gpsimd.partition_broadcast(bc, rec)
                nc.vector.tensor_mul(asl, asl, bc)

            # proj slices feed attention tiles; outproj quanta of tile t are
            # queued when attn(t+1) starts and drained inside the chunk loops
            _proj(0)
            _attn(0)
            _proj(1)
            _attn(1)
            opq.extend(_outproj_quanta(0))
            _attn(2)
            opq.extend(_outproj_quanta(1))
            _proj(2)
            _attn(3)
            opq.extend(_outproj_quanta(2))
            _attn(4)
            opq.extend(_outproj_quanta(3))
            _proj(3)
            _attn(5)
            opq.extend(_outproj_quanta(4))
            _attn(6)
            opq.extend(_outproj_quanta(5))
            _attn(7)
            opq.extend(_outproj_quanta(6))
            opq.extend(_outproj_quanta(7))
            while opq:
                opq.pop(0)()

    nc.finalize()
    return nc


def make_host_inputs(x, Wq, Wkv, Wout):
    """Shard + precompute per-core input maps (host side)."""
    B, N, DIM = x.shape
    bf = ml_dtypes.bfloat16
    xTb = [np.ascontiguousarray(x[b].T).astype(bf) for b in range(B)]
    inv = 1.0 / (10000.0 ** (np.arange(0, D, 2, dtype=np.float64) / D))
    fr = np.arange(N, dtype=np.float64)[:, None] * inv[None, :]
    pos = np.concatenate([fr, fr], axis=-1)              # [N, D]
    cos_t = np.cos(pos).T.astype(np.float32)             # [D, N]
    sin_t = np.sin(pos).T.astype(np.float32)
    sign = np.ones((D, 1), np.float32)
    sign[:D // 2] = -1.0
    sin_r = sin_t * sign            # fold rotate_half's sign into the table
    # pre-rotate: row p holds sin_signed[(p+64)%128] so the kernel's
    # same-base-partition reads line up (see _rope)
    sin_r = np.roll(sin_r, -D // 2, axis=0)
    shared = dict(
        wkv=Wkv.astype(bf),
        cosq=np.ascontiguousarray(cos_t * SCALE).astype(bf),
        sinq=np.ascontiguousarray(sin_r * SCALE).astype(bf),
        cosk=cos_t.astype(bf), sink=sin_r.astype(bf))
    in_maps = []
    for c in range(N_CORES):
        b = c // GROUPS
        hg = c % GROUPS
        lo, hi = hg * HL * D, (hg + 1) * HL * D
        in_maps.append(dict(
            shared,
            xT=xTb[b],
            wq=np.ascontiguousarray(Wq[:, lo:hi]).astype(bf),
            wout=np.ascontiguousarray(Wout[lo:hi, :]).astype(bf)))
    return in_maps


def kernel(x, Wq, Wkv, Wout):
    B, N, DIM = x.shape
    nc = build_nc(N, DIM)
    in_maps = make_host_inputs(x, Wq, Wkv, Wout)
    res = run_bass_kernel_spmd(nc, in_maps, core_ids=list(range(N_CORES)))
    y = np.zeros((B, N, DIM), np.float32)
    for c, r in enumerate(res.results):
        y[c // GROUPS] += r["y"].astype(np.float32)
    return y


# revision 40
# speedup vs baseline: 1.0985x; 1.0603x over previous
"""Trainium2 Bass kernel for MQA causal attention (16 q heads, 1 shared kv head).

Sharding: hybrid batch x tensor-parallel. Core c handles batch c//4 and query
heads [4*(c%4), 4*(c%4)+4) (4 heads per core). Shared K/V is sharded: each
core computes one quarter of the sequence's k/v (host passes the x slice +
rope-table slice so the SPMD program stays core-independent) and the batch
group of 4 cores AllGathers the roped K / transposed V through DRAM; the
collective for rep r+1 is launched at the start of rep r so it hides behind
a full body of compute. Each core emits a bf16 partial out-projection for
its batch; the host sums the 4 partials per batch (the all-reduce of the
hint).

Per-core layout:
  - x arrives dim-major (xT, bf16): every matmul contraction dim is already
    on partitions; no on-chip transposes of x.
  - Projections: psq/psk/psv accumulate over DC=16 chunks in PSUM, then are
    copied to SBUF bf16 on ACT; RoPE runs on DVE fully in bf16 (2x mode):
    q_rot = q*cos + rot(q)*sin with rot done by partition-offset reads and
    host-pre-signed sin tables. q tables pre-scaled by 1/sqrt(d).
  - Attention is computed transposed at 4-head width: simT[keys, h*q] =
    kT.T @ qT per 128-key chunk, 2 matmuls (head pairs) so every matmul
    output stays within one PSUM bank. exp on ACT ([128,1024] per op),
    causal masking only on the two diagonal key chunks via affine_select,
    denominator = ones-column matmuls accumulated in PSUM, attn@V keeps V
    natural [keys, d] (PE-transposed at projection time) accumulating
    psa[d, h*q] in PSUM.
  - psa is evicted UN-normalized (frees the single psa PSUM ring slot
    early); normalization happens in-place on the bf16 tile after a
    reciprocal + partition_broadcast of the denominators.
  - Out-projection: attnT chunks stationary, Wout slice moving; psy evicted
    to bf16 ysb split across DVE and ACT; y written bf16.
  - Emission interleaves projection slices, attention tiles and
    out-projection chunks so the PE stream fills ACT-wait gaps.
"""

import os
import sys
from contextlib import ExitStack

import numpy as np

for _p in ("/opt/trn_rl_repo",):
    if os.path.isdir(_p) and _p not in sys.path:
        sys.path.insert(0, _p)

import ml_dtypes

import concourse.bass as bass
import concourse.mybir as mybir
import concourse.tile as tile
from concourse import bacc
from concourse.bass_utils import run_bass_kernel_spmd
from concourse.masks import make_identity

HEADS = 16
D = 128
SCALE = D ** -0.5
N_CORES = 8
HL = 4                      # query heads per core
GROUPS = 4                  # cores per batch group

F32 = mybir.dt.float32
BF16 = mybir.dt.bfloat16


def _rope(nc, sb, src, out_slice, cos_s, sin_s):
    """out_slice(bf16) = src*cos_s + rot(src)*sin_s, all bf16 on DVE (2x).

    sin_s arrives pre-signed AND pre-rotated from the host (halves swapped,
    rows that multiply the swapped-in half negated) so that both DVE inputs
    always share the same base partition (a same-space DVE requirement)."""
    L = src.shape[-1]
    t1 = sb.tile([128, L], BF16, tag="ropet1")
    nc.vector.tensor_mul(t1, src, cos_s)
    t2 = sb.tile([128, L], BF16, tag="ropet2")
    nc.vector.tensor_mul(t2[0:64, :], src[64:128, :], sin_s[64:128, :])
    nc.vector.tensor_mul(t2[64:128, :], src[0:64, :], sin_s[0:64, :])
    nc.vector.tensor_add(out_slice, t1, t2)


def build_nc(N, DIM, reps=1):
    """One SPMD program: HL query heads + shared kv head, one batch,
    full sequence. reps>1 repeats the body for timing-by-difference."""
    DC = DIM // 128           # contraction chunks for projections
    SL = 512                  # projection n-slice length
    NS = N // SL              # n slices (4)
    NKC = N // 128            # 128-wide key chunks (16)
    NQT = N // 256            # 256-row query tiles (8)
    KPS = SL // 128           # key chunks per slice (4)

    nc = bacc.Bacc(None, target_bir_lowering=False)
    xT = nc.declare_dram_parameter("xT", [DIM, N], BF16, isOutput=False)
    # this core's kv shard: xkv = xT[:, hg*512:(hg+1)*512] host-sliced, so
    # the SPMD program stays core-id independent; k rope tables likewise
    xkv = nc.declare_dram_parameter("xkv", [DIM, SL], BF16, isOutput=False)
    ckv = nc.declare_dram_parameter("ckv", [D, SL], BF16, isOutput=False)
    skv = nc.declare_dram_parameter("skv", [D, SL], BF16, isOutput=False)
    wq = nc.declare_dram_parameter("wq", [DIM, HL * D], BF16, isOutput=False)
    wkv = nc.declare_dram_parameter("wkv", [DIM, 2 * D], BF16, isOutput=False)
    wout = nc.declare_dram_parameter("wout", [HL * D, DIM], BF16, isOutput=False)
    cosq = nc.declare_dram_parameter("cosq", [D, N], BF16, isOutput=False)
    sinq = nc.declare_dram_parameter("sinq", [D, N], BF16, isOutput=False)
    y = nc.declare_dram_parameter("y", [N, DIM], BF16, isOutput=True)
    REPLICA_GROUPS = [[0, 1, 2, 3], [4, 5, 6, 7]]

    with ExitStack() as ctx:
        tc = ctx.enter_context(tile.TileContext(nc))
        consts = ctx.enter_context(tc.tile_pool(name="consts", bufs=1))
        xpool = ctx.enter_context(tc.tile_pool(name="xpool", bufs=2))
        proj = ctx.enter_context(tc.tile_pool(name="proj", bufs=2))
        sb = ctx.enter_context(tc.tile_pool(name="sb", bufs=2))
        misc = ctx.enter_context(tc.tile_pool(name="misc", bufs=2))
        dram = ctx.enter_context(tc.tile_pool(name="dram", bufs=2, space="DRAM"))
        # PSUM: stream ring 3 x [128,1024]f32 (6 banks) for all transient
        # psums (sim chunks, proj groups, outproj psy, per-tile psd) + psa
        # ring 1 (2 banks) = 8 banks.
        ps_str = ctx.enter_context(tc.tile_pool(name="ps_str", bufs=3, space="PSUM"))
        ps_acc = ctx.enter_context(tc.tile_pool(name="ps_acc", bufs=1, space="PSUM"))

        ident = consts.tile([128, 128], BF16)
        make_identity(nc, ident)
        ones_col = consts.tile([128, 1], BF16)
        nc.vector.memset(ones_col, 1.0)

        # only SP (sync) and ACT (scalar) have HWDGE queues; order for ramp:
        # sync: wkv, x slice 0, k tables, x slices 1-3
        # scalar: wq, q tables, wout, then y writes
        wq_sb = consts.tile([128, DC, HL * D], BF16)
        wkv_sb = consts.tile([128, DC, 2 * D], BF16)
        nc.sync.dma_start(wkv_sb, wkv.rearrange("(c p) m -> p c m", p=128))
        nc.scalar.dma_start(wq_sb, wq.rearrange("(c p) m -> p c m", p=128))
        cq_sb = consts.tile([128, N], BF16)
        sq_sb = consts.tile([128, N], BF16)
        ckv_sb = consts.tile([128, SL], BF16)
        skv_sb = consts.tile([128, SL], BF16)
        nc.sync.dma_start(ckv_sb, ckv[:, :])
        nc.sync.dma_start(skv_sb, skv[:, :])
        wout_sb = consts.tile([128, HL, DIM], BF16)

        tables_loaded = [False]

        def _load_tables():
            # emitted after x slice 0's dma so the slice-0 stream goes first
            nc.scalar.dma_start(cq_sb, cosq[:, :])
            nc.scalar.dma_start(sq_sb, sinq[:, :])
            nc.scalar.dma_start(
                wout_sb, wout.rearrange("(c p) m -> p c m", p=128))
            tables_loaded[0] = True

        def _kv_launch():
            """Compute this core's quarter of k/v (+rope/transpose), stage
            to DRAM and trigger the 4-core AllGather. Returns the gathered
            DRAM tile. Stage-out + trigger ride the Pool queue."""
            xkv_sb = xpool.tile([128, DC, SL], BF16, tag="xt")
            xkv_src = xkv.rearrange("(c p) n -> p c n", p=128)
            nc.sync.dma_start(xkv_sb[:, :DC // 2, :], xkv_src[:, :DC // 2, :])
            nc.sync.dma_start(xkv_sb[:, DC // 2:, :], xkv_src[:, DC // 2:, :])
            psv = ps_str.tile([128, SL], F32, tag="stream")
            for dc in range(DC):
                nc.tensor.matmul(
                    psv, wkv_sb[:, dc, D:2 * D], xkv_sb[:, dc, :],
                    start=(dc == 0), stop=(dc == DC - 1))
            vt_sb = sb.tile([128, SL], BF16, tag="vt")
            nc.scalar.copy(vt_sb, psv)
            psk = ps_str.tile([128, SL], F32, tag="stream")
            for dc in range(DC):
                nc.tensor.matmul(
                    psk, wkv_sb[:, dc, 0:D], xkv_sb[:, dc, :],
                    start=(dc == 0), stop=(dc == DC - 1))
            ks = sb.tile([128, SL], BF16, tag="ks")
            nc.scalar.copy(ks, psk)
            kloc = sb.tile([128, SL], BF16, tag="kloc")
            _rope(nc, sb, ks, kloc, ckv_sb, skv_sb)
            vloc = sb.tile([128, KPS, D], BF16, tag="vloc")
            for kc in range(KPS):
                pst = ps_str.tile([128, 128], BF16, tag="stream")
                nc.tensor.transpose(pst, vt_sb[:, kc * 128:(kc + 1) * 128], ident)
                nc.vector.tensor_copy(vloc[:, kc, :], pst)
            kv_loc = dram.tile([128, 2 * SL], BF16, tag="kv_loc")
            kv_all = dram.tile([GROUPS, 128, 2 * SL], BF16, tag="kv_all")
            nc.gpsimd.dma_start(kv_loc[:, 0:SL], kloc)
            nc.gpsimd.dma_start(kv_loc[:, SL:2 * SL], vloc)
            nc.gpsimd.collective_compute(
                "AllGather", mybir.AluOpType.bypass, REPLICA_GROUPS,
                ins=[kv_loc[:, :]], outs=[kv_all[:, :, :]])
            return kv_all

        def _kv_land(kv_all, krot, vnat):
            """Gathered k/v -> SBUF. On the sync queue, emitted LATE in the
            prior rep so the collective has completed and never blocks the
            queue head in front of the next rep's x-tile prefetches."""
            for g in range(GROUPS):
                nc.sync.dma_start(
                    krot[:, g * SL:(g + 1) * SL], kv_all[g, :, 0:SL])
                nc.sync.dma_start(
                    vnat[:, g * KPS:(g + 1) * KPS, :],
                    kv_all[g, :, SL:2 * SL])

        pending = None   # (kv_all, krot_next, vnat_next) launched a rep ago

        for rep in range(reps):
            first = rep == 0
            qrot = proj.tile([128, HL, N], BF16, tag="qrot")
            attnT = proj.tile([128, HL, N], BF16, tag="attnT")
            if pending is None:
                krot = proj.tile([128, N], BF16, tag="krot")
                vnat = proj.tile([128, NKC, D], BF16, tag="vnat")
                _kv_land(_kv_launch(), krot, vnat)
            else:
                _, krot, vnat = pending
            if rep + 1 < reps:
                kv_all2 = _kv_launch()
                krot2 = proj.tile([128, N], BF16, tag="krot")
                vnat2 = proj.tile([128, NKC, D], BF16, tag="vnat")
                pending = (kv_all2, krot2, vnat2)
            else:
                pending = None

            def _proj(s, qrot=qrot, first=first):
                sl = slice(s * SL, (s + 1) * SL)
                xt = xpool.tile([128, DC, SL], BF16, tag="xt")
                h_dc = DC // 2
                xt_src = xT.rearrange("(c p) n -> p c n", p=128)[:, :, sl]
                nc.sync.dma_start(xt[:, :h_dc, :], xt_src[:, :h_dc, :])
                nc.sync.dma_start(xt[:, h_dc:, :], xt_src[:, h_dc:, :])
                if not tables_loaded[0]:
                    _load_tables()
                for h in range(HL):
                    psq = ps_str.tile([128, SL], F32, tag="stream")
                    for dc in range(DC):
                        nc.tensor.matmul(
                            psq, wq_sb[:, dc, h * D:(h + 1) * D], xt[:, dc, :],
                            start=(dc == 0), stop=(dc == DC - 1))
                    qs = sb.tile([128, SL], BF16, tag="qs")
                    nc.scalar.copy(qs, psq)
                    _rope(nc, sb, qs, qrot[:, h, sl], cq_sb[:, sl], sq_sb[:, sl])

            # outproj work queue: each entry is a closure emitting ~0.9us of
            # PE work (half a psy group); drained one per attention chunk so
            # the out-projection fills the PE slack while ACT runs exp.
            opq = []

            def _outproj_quanta(t, attnT=attnT):
                for m in (2 * t, 2 * t + 1):
                    for nso in range(DIM // 1024):
                        state = {}

                        def q1(m=m, nso=nso, state=state):
                            psy = ps_str.tile([128, 1024], F32, tag="stream",
                                              name="psy")
                            state["psy"] = psy
                            for hc in range(HL):
                                nc.tensor.matmul(
                                    psy[:, 0:512],
                                    attnT[:, hc, m * 128:(m + 1) * 128],
                                    wout_sb[:, hc,
                                            nso * 1024:nso * 1024 + 512],
                                    start=(hc == 0), stop=(hc == HL - 1))

                        def q2(m=m, nso=nso, state=state):
                            psy = state["psy"]
                            for hc in range(HL):
                                nc.tensor.matmul(
                                    psy[:, 512:1024],
                                    attnT[:, hc, m * 128:(m + 1) * 128],
                                    wout_sb[:, hc,
                                            nso * 1024 + 512:(nso + 1) * 1024],
                                    start=(hc == 0), stop=(hc == HL - 1))
                            ysb = misc.tile([128, 1024], BF16, tag="ysb")
                            if nso % 2 == 0:
                                nc.vector.tensor_copy(ysb, psy)
                            else:
                                nc.scalar.copy(ysb, psy)
                            nc.scalar.dma_start(
                                y[m * 128:(m + 1) * 128,
                                  nso * 1024:(nso + 1) * 1024], ysb)

                        yield q1
                        yield q2

            def _attn(t, qrot=qrot, krot=krot, vnat=vnat, attnT=attnT):
                nkc = 2 * t + 2
                psa = ps_acc.tile([128, HL, 256], F32, tag="psa")
                exacc = sb.tile([128, HL, 256], BF16, tag="exacc")
                qsl = qrot[:, :, t * 256:(t + 1) * 256]
                # diagonal chunks FIRST: their Pool mask latency then hides
                # behind the long run of mask-free chunks instead of stalling
                # the PE at the tile boundary.  attnv of chunk j is emitted
                # AFTER sim of chunk j+1 (software pipelining) so the
                # in-order PE stream never head-of-line blocks on ACT's exp.
                order = [2 * t, 2 * t + 1] + list(range(0, 2 * t))

                def _consume(ci, j, ex):
                    st, sp = ci == 0, ci == nkc - 1
                    nc.tensor.matmul(psa[:, 0:2, :], vnat[:, j, :], ex[:, 0:2, :],
                                     start=st, stop=sp)
                    nc.tensor.matmul(psa[:, 2:4, :], vnat[:, j, :], ex[:, 2:4, :],
                                     start=st, stop=sp)
                    # denominator accumulation rides DVE (bf16 2x), not PE
                    if ci == 0:
                        nc.vector.tensor_copy(exacc, ex)
                    else:
                        nc.vector.tensor_add(exacc, exacc, ex)

                prev = None
                for ci, j in enumerate(order):
                    kj = krot[:, j * 128:(j + 1) * 128]
                    pss = ps_str.tile([128, HL, 256], F32, tag="stream")
                    nc.tensor.matmul(pss[:, 0:2, :], kj, qsl[:, 0:2, :],
                                     start=True, stop=True)
                    nc.tensor.matmul(pss[:, 2:4, :], kj, qsl[:, 2:4, :],
                                     start=True, stop=True)
                    ex = sb.tile([128, HL, 256], BF16, tag="ex", bufs=3)
                    nc.scalar.activation(ex, pss, mybir.ActivationFunctionType.Exp)
                    if j >= 2 * t:
                        # diagonal chunk: keep where q - p - base >= 0
                        nc.gpsimd.affine_select(
                            out=ex, in_=ex,
                            compare_op=mybir.AluOpType.is_ge, fill=0.0,
                            base=(0 if j == 2 * t else -128),
                            pattern=[[0, HL], [1, 256]],
                            channel_multiplier=-1)
                    if prev is not None:
                        _consume(*prev)
                    prev = (ci, j, ex)
                    if ci >= 2 and opq:
                        opq.pop(0)()
                _consume(*prev)
                # denominator: one PE reduction of exacc per tile, into a
                # transient stream-slot psd; then recip+broadcast+normalize
                psd = ps_str.tile([1, HL, 256], F32, tag="stream", name="psd")
                nc.tensor.matmul(psd[:, 0:2, :], ones_col, exacc[:, 0:2, :],
                                 start=True, stop=True)
                nc.tensor.matmul(psd[:, 2:4, :], ones_col, exacc[:, 2:4, :],
                                 start=True, stop=True)
                # evict unnormalized (frees psa ring slot), normalize in place
                asl = attnT[:, :, t * 256:(t + 1) * 256]
                nc.vector.tensor_copy(asl, psa)
                rec = misc.tile([1, HL, 256], BF16, tag="rec")
                with nc.allow_low_precision("softmax recip in bf16 is ~0.1%"):
                    nc.vector.reciprocal(rec, psd)
                bc = misc.tile([128, HL, 256], BF16, tag="bc")
                nc.gpsimd.partition_broadcast(bc, rec)
                nc.vector.tensor_mul(asl, asl, bc)

            # proj slices feed attention tiles; outproj quanta of tile t are
            # queued when attn(t+1) starts and drained inside the chunk loops
            _proj(0)
            _attn(0)
            _proj(1)
            _attn(1)
            opq.extend(_outproj_quanta(0))
            _attn(2)
            opq.extend(_outproj_quanta(1))
            _proj(2)
            _attn(3)
            opq.extend(_outproj_quanta(2))
            _attn(4)
            opq.extend(_outproj_quanta(3))
            _proj(3)
            _attn(5)
            opq.extend(_outproj_quanta(4))
            _attn(6)
            if pending is not None:
                # land next rep's gathered k/v now: the collective (launched
                # at this rep's start) is long done, so these never block
                # the sync queue
                _kv_land(*pending)
            opq.extend(_outproj_quanta(5))
            _attn(7)
            opq.extend(_outproj_quanta(6))
            opq.extend(_outproj_quanta(7))
            while opq:
                opq.pop(0)()

    nc.finalize()
    return nc


def make_host_inputs(x, Wq, Wkv, Wout):
    """Shard + precompute per-core input maps (host side)."""
    B, N, DIM = x.shape
    bf = ml_dtypes.bfloat16
    xTb = [np.ascontiguousarray(x[b].T).astype(bf) for b in range(B)]
    inv = 1.0 / (10000.0 ** (np.arange(0, D, 2, dtype=np.float64) / D))
    fr = np.arange(N, dtype=np.float64)[:, None] * inv[None, :]
    pos = np.concatenate([fr, fr], axis=-1)              # [N, D]
    cos_t = np.cos(pos).T.astype(np.float32)             # [D, N]
    sin_t = np.sin(pos).T.astype(np.float32)
    sign = np.ones((D, 1), np.float32)
    sign[:D // 2] = -1.0
    sin_r = sin_t * sign            # fold rotate_half's sign into the table
    # pre-rotate: row p holds sin_signed[(p+64)%128] so the kernel's
    # same-base-partition reads line up (see _rope)
    sin_r = np.roll(sin_r, -D // 2, axis=0)
    SL = N // GROUPS
    shared = dict(
        wkv=Wkv.astype(bf),
        cosq=np.ascontiguousarray(cos_t * SCALE).astype(bf),
        sinq=np.ascontiguousarray(sin_r * SCALE).astype(bf))
    cosk_b = cos_t.astype(bf)
    sink_b = sin_r.astype(bf)
    in_maps = []
    for c in range(N_CORES):
        b = c // GROUPS
        hg = c % GROUPS
        lo, hi = hg * HL * D, (hg + 1) * HL * D
        ksl = slice(hg * SL, (hg + 1) * SL)
        in_maps.append(dict(
            shared,
            xT=xTb[b],
            xkv=np.ascontiguousarray(xTb[b][:, ksl]),
            ckv=np.ascontiguousarray(cosk_b[:, ksl]),
            skv=np.ascontiguousarray(sink_b[:, ksl]),
            wq=np.ascontiguousarray(Wq[:, lo:hi]).astype(bf),
            wout=np.ascontiguousarray(Wout[lo:hi, :]).astype(bf)))
    return in_maps


def kernel(x, Wq, Wkv, Wout):
    B, N, DIM = x.shape
    nc = build_nc(N, DIM)
    in_maps = make_host_inputs(x, Wq, Wkv, Wout)
    res = run_bass_kernel_spmd(nc, in_maps, core_ids=list(range(N_CORES)))
    y = np.zeros((B, N, DIM), np.float32)
    for c, r in enumerate(res.results):
        y[c // GROUPS] += r["y"].astype(np.float32)
    return y
